# revision 1
# baseline (speedup 1.0000x reference)
"""Trainium2 Bass kernel for nn_ClusterPrediction (DynamicEdgeConv x3 + edge head).

Math (reference):
  3x DynamicEdgeConv: kNN(k=30) in feature space, per-edge MLP on
  [x_i, x_j - x_i] with LeakyReLU(0.2), max aggregation.
  Edge head on canonicalized (sorted) edge_index columns:
  sigmoid(W2 . LRelu(Wh1 . [x_u, x_v] + bh1) + bh2).

Key device-side tricks:
  * Distance ranking key: S_ij = 2 x_i.x_j - |x_j|^2 (row-constant |x_i|^2
    dropped). Computed as ONE matmul with augmented contraction:
    stationary [2*x_q ; 1], moving [x_all ; -|x|^2] (streamed from DRAM in
    512-column chunks).
  * Top-30 via DVE max/max_index/match_replace (top-8 per call):
    - per 512-chunk: pack the chunk-local index (9 bits) into the low
      mantissa bits of the fp32 key (scalar_tensor_tensor:
      (S & 0xFFFFFE00) | iota), then vector.max -> per-chunk top-8
      candidates whose values carry their own indices.
    - phase 2 on the 32*8 candidates: 4 rounds of max / max_index /
      match_replace -> ranks 1..32 descending; ranks 31/32 replaced by a
      duplicate of rank 1 (no-op under max-aggregation).
    - global index = (candidate_pos >> 3)*512 + (bits & 0x1FF).
  * MLP decomposed: W.[x_i ; x_j - x_i] = (Wa-Wb).x_i + Wb.x_j; LeakyReLU
    is monotonic and the +v_i term commutes with max, so
    out_i = LRelu(v_i + b + max_k u_{j_k}) with u = Wb.x. Per-neighbor work
    is just a gather of u^T columns (gpsimd.ap_gather, on-chip) + max.
  * Features sharded over 8 cores (2048 queries each); AllGather of x
    (transposed) between layers.
"""

import numpy as np

import concourse.bacc as bacc
import concourse.bass as bass
import concourse.mybir as mybir
import concourse.tile as tile
from concourse.bass_utils import run_bass_kernel_spmd

FP = mybir.dt.float32
FPR = mybir.dt.float32r
U32 = mybir.dt.uint32
I16 = mybir.dt.int16
AX = mybir.AxisListType
ALU = mybir.AluOpType
ACTF = mybir.ActivationFunctionType

N_CORES = 8
K = 30
KPAD = 32          # 4 rounds of top-8
NEG = 0.2          # LeakyReLU slope
CH = 512           # distance chunk (columns per PSUM bank)
BSET = 4           # query blocks processed per chunk-sweep


def build_program(N=16384, E=262144, n_cores=N_CORES):
    NQ = N // n_cores            # queries per core
    NB = NQ // 128               # 128-query blocks per core
    ch = min(CH, NQ)             # distance chunk columns
    NCH = N // ch                # chunks per distance row
    NCAND = NCH * 8              # candidates per row
    EC = E // n_cores            # edges per core
    ECH = min(2048, EC)          # edges per gather chunk
    NECH = EC // ECH
    NSET = max(1, NB // BSET)
    BS = NB // NSET              # blocks per set
    assert NQ % 128 == 0 and N % ch == 0 and EC % ECH == 0 and ECH % 512 == 0
    assert NB % NSET == 0

    nc = bacc.Bacc("TRN2", target_bir_lowering=False, num_devices=n_cores)

    # ---------------- I/O ----------------
    m1_d = nc.dram_tensor("m1", [4, N], FP, kind="ExternalInput")
    stat1_d = nc.dram_tensor("stat1", [4, NQ], FP, kind="ExternalInput")
    wd1_d = nc.dram_tensor("wd1", [3, 64], FP, kind="ExternalInput")
    wb1_d = nc.dram_tensor("wb1", [3, 64], FP, kind="ExternalInput")
    b1_d = nc.dram_tensor("b1t", [64, 1], FP, kind="ExternalInput")
    wd2_d = nc.dram_tensor("wd2", [64, 64], FP, kind="ExternalInput")
    wb2_d = nc.dram_tensor("wb2", [64, 64], FP, kind="ExternalInput")
    b2_d = nc.dram_tensor("b2t", [64, 1], FP, kind="ExternalInput")
    wd3_d = nc.dram_tensor("wd3", [64, 64], FP, kind="ExternalInput")
    wb3_d = nc.dram_tensor("wb3", [64, 64], FP, kind="ExternalInput")
    b3_d = nc.dram_tensor("b3t", [64, 1], FP, kind="ExternalInput")
    wha_d = nc.dram_tensor("wha", [64, 64], FP, kind="ExternalInput")
    whb_d = nc.dram_tensor("whb", [64, 64], FP, kind="ExternalInput")
    bh1_d = nc.dram_tensor("bh1t", [64, 1], FP, kind="ExternalInput")
    w2_d = nc.dram_tensor("w2t", [64, 1], FP, kind="ExternalInput")
    bh2_d = nc.dram_tensor("bh2t", [1, 1], FP, kind="ExternalInput")
    ew0_d = nc.dram_tensor("ew0", [64, EC // 16], I16, kind="ExternalInput")
    ew1_d = nc.dram_tensor("ew1", [64, EC // 16], I16, kind="ExternalInput")
    out_d = nc.dram_tensor("out", [EC], FP, kind="ExternalOutput")

    with tile.TileContext(nc, num_cores=n_cores) as tc:
        with (
            tc.tile_pool(name="const", bufs=1) as cpool,
            tc.tile_pool(name="mchunk", bufs=4) as mcpool,
            tc.tile_pool(name="prep", bufs=3) as ppool,
            tc.tile_pool(name="xpool", bufs=1) as xpool,
            tc.tile_pool(name="upool", bufs=1) as upool,
            tc.tile_pool(name="cpoolc", bufs=2) as candpool,
            tc.tile_pool(name="pk", bufs=3) as pkpool,
            tc.tile_pool(name="small", bufs=3) as spool,
            tc.tile_pool(name="gat", bufs=4) as gpool,
            tc.tile_pool(name="dram", bufs=2, space="DRAM") as dpool,
            tc.tile_pool(name="pdist", bufs=3, space="PSUM") as pdist,
            tc.tile_pool(name="pmid", bufs=4, space="PSUM") as pmid,
        ):
            # ------------ load constants ------------
            def load(dram, shape, dtype=FP):
                t = cpool.tile(shape, dtype, tag=f"c_{dram.name}")
                nc.sync.dma_start(t[:], dram[:])
                return t

            stat1t = load(stat1_d, [4, NQ])
            wd = [load(wd1_d, [3, 64]), load(wd2_d, [64, 64]), load(wd3_d, [64, 64])]
            wb = [load(wb1_d, [3, 64]), load(wb2_d, [64, 64]), load(wb3_d, [64, 64])]
            bt = [load(b1_d, [64, 1]), load(b2_d, [64, 1]), load(b3_d, [64, 1])]
            what = load(wha_d, [64, 64])
            whbt = load(whb_d, [64, 64])
            bh1t = load(bh1_d, [64, 1])
            w2t = load(w2_d, [64, 1])
            bh2t = load(bh2_d, [1, 1])
            ew0t = load(ew0_d, [64, EC // 16], I16)
            ew1t = load(ew1_d, [64, EC // 16], I16)

            iota9 = cpool.tile([128, ch], U32)
            nc.gpsimd.iota(iota9[:], pattern=[[1, ch]], base=0, channel_multiplier=0)
            mask_hi = cpool.tile([128, 1], U32)
            nc.gpsimd.memset(mask_hi[:], 0xFFFFFE00)
            mask_lo = cpool.tile([128, 1], U32)
            nc.gpsimd.memset(mask_lo[:], 0x1FF)
            mask_p8 = cpool.tile([128, 1], U32)
            nc.gpsimd.memset(mask_p8[:], 0xFFFFFFF8)
            ones64 = cpool.tile([64, 1], FP)
            nc.gpsimd.memset(ones64[:], 1.0)

            cur_STAT = None   # [65, NQ] rows 0..63 2*x^T(local), row 64 ones
            cur_ccout = None  # DRAM [8*64, NQ], previous layer's AllGather
            XLT = None

            def conv_layer(li):
                nonlocal cur_STAT, cur_ccout, XLT
                Cin = 3 if li == 0 else 64
                KD = Cin + 1
                Ssrc = stat1t if li == 0 else cur_STAT

                # ---- pre-pass: u^T = wb^T.x^T [64, N]; (l>=2) -|x|^2 row ----
                UT = upool.tile([64, N], FP, tag="ut")
                sqrow = None
                if li > 0:
                    sqrow = dpool.tile([1, N], FP, tag="sqr")
                for c in range(NCH):
                    cs = slice(c * ch, (c + 1) * ch)
                    if li == 0:
                        trows = ppool.tile([3, ch], FP, tag="trows")
                        nc.sync.dma_start(trows[:], m1_d[0:3, cs])
                    else:
                        trows = ppool.tile([64, ch], FP, tag="trows")
                        rk, lc = (c * ch) // NQ, (c * ch) % NQ
                        nc.sync.dma_start(
                            trows[:],
                            cur_ccout[rk * 64:(rk + 1) * 64, lc:lc + ch],
                        )
                    pu = pmid.tile([64, ch], FP, tag="pmid")
                    nc.tensor.matmul(
                        pu[:], wb[li][:], trows[0:Cin, :]
                    )
                    nc.scalar.copy(UT[:, cs], pu[:])
                    if li > 0:
                        sqc = ppool.tile([64, ch], FP, tag="sqc")
                        nc.scalar.square(sqc[:], trows[:])
                        po = pmid.tile([1, ch], FP, tag="pmid")
                        nc.tensor.matmul(
                            po[:], ones64[:], sqc[:]
                        )
                        sqsb = ppool.tile([1, ch], FP, tag="sqsb")
                        nc.scalar.mul(sqsb[:], po[:], -1.0)
                        nc.sync.dma_start(sqrow[0:1, cs], sqsb[:])

                XLT_new = xpool.tile([64, NQ], FP, tag="xlt")
                for st in range(NSET):
                    blocks = range(st * BS, (st + 1) * BS)
                    cand = candpool.tile([128, BS * NCAND], FP, tag="cand")
                    # ---- distance sweep: chunks outer, blocks inner ----
                    for c in range(NCH):
                        cs = slice(c * ch, (c + 1) * ch)
                        mch = mcpool.tile([KD, ch], FP, tag="mch")
                        if li == 0:
                            nc.sync.dma_start(mch[:], m1_d[:, cs])
                        else:
                            rk, lc = (c * ch) // NQ, (c * ch) % NQ
                            nc.sync.dma_start(
                                mch[0:64, :],
                                cur_ccout[rk * 64:(rk + 1) * 64, lc:lc + ch],
                            )
                            nc.sync.dma_start(mch[64:65, :], sqrow[0:1, cs])
                        for bi, b in enumerate(blocks):
                            bs_ = slice(b * 128, (b + 1) * 128)
                            pd = pdist.tile([128, ch], FP, tag="pd")
                            nc.tensor.matmul(
                                pd[:],
                                Ssrc[0:KD, bs_],
                                mch[:],
                            )
                            pk = pkpool.tile([128, ch], U32, tag="pk")
                            nc.vector.scalar_tensor_tensor(
                                pk[:],
                                pd[:].bitcast(U32),
                                mask_hi[:],
                                iota9[:],
                                op0=ALU.bitwise_and,
                                op1=ALU.bitwise_or,
                            )
                            nc.vector.max(
                                cand[:, bi * NCAND + c * 8: bi * NCAND + c * 8 + 8],
                                pk[:].bitcast(FP),
                            )

                    # ---- per block: select, decode, gather, aggregate ----
                    for bi, b in enumerate(blocks):
                        bs_ = slice(b * 128, (b + 1) * 128)
                        cb = cand[:, bi * NCAND:(bi + 1) * NCAND]
                        wv = spool.tile([128, KPAD], FP, tag="wv")
                        pos = spool.tile([128, KPAD], U32, tag="pos")
                        for r in range(4):
                            rs = slice(r * 8, (r + 1) * 8)
                            nc.vector.max(wv[:, rs], cb)
                            nc.vector.max_index(pos[:, rs], wv[:, rs], cb)
                            if r < 3:
                                nc.vector.match_replace(cb, wv[:, rs], cb, -3.0e38)

                        posm = spool.tile([128, KPAD], U32, tag="posm")
                        nc.vector.scalar_tensor_tensor(
                            posm[:], pos[:], mask_p8[:], pos[:],
                            op0=ALU.bitwise_and, op1=ALU.bypass,
                        )
                        posf = spool.tile([128, KPAD], FP, tag="posf")
                        nc.vector.tensor_copy(posf[:], posm[:])
                        lom = spool.tile([128, KPAD], U32, tag="lom")
                        nc.vector.scalar_tensor_tensor(
                            lom[:], wv[:].bitcast(U32), mask_lo[:],
                            wv[:].bitcast(U32),
                            op0=ALU.bitwise_and, op1=ALU.bypass,
                        )
                        lof = spool.tile([128, KPAD], FP, tag="lof")
                        nc.vector.tensor_copy(lof[:], lom[:])
                        idxf = spool.tile([128, KPAD], FP, tag="idxf")
                        nc.vector.scalar_tensor_tensor(
                            idxf[:], posf[:], float(ch // 8), lof[:],
                            op0=ALU.mult, op1=ALU.add,
                        )
                        # ranks 31/32 are outside the top-30: duplicate rank 1
                        nc.vector.tensor_copy(idxf[:, 30:31], idxf[:, 0:1])
                        nc.vector.tensor_copy(idxf[:, 31:32], idxf[:, 0:1])
                        idx16 = spool.tile([128, KPAD], I16, tag="idx16")
                        nc.vector.tensor_copy(idx16[:], idxf[:])

                        # rewrap for ap_gather via DRAM bounce:
                        # element i = p*32+k -> (partition i%16, col i//16)
                        sc = dpool.tile([128, KPAD], I16, tag="scidx")
                        nc.sync.dma_start(sc[:], idx16[:])
                        wrap = spool.tile([64, KPAD * 8], I16, tag="wrap")
                        src = sc[:].rearrange("p (kh q) -> q p kh", q=16)
                        for g in range(4):
                            dst = wrap[g * 16:(g + 1) * 16, :].rearrange(
                                "q (p kh) -> q p kh", kh=2
                            )
                            nc.sync.dma_start(dst, src)

                        mT = spool.tile([64, 128], FP, tag="mT")
                        for h in range(2):
                            gath = gpool.tile([64, 64 * KPAD], FP, tag="gath")
                            nc.gpsimd.ap_gather(
                                gath[:], UT[:],
                                wrap[:, h * 128:(h + 1) * 128],
                                channels=64, num_elems=N, d=1,
                                num_idxs=64 * KPAD,
                            )
                            nc.vector.tensor_reduce(
                                mT[:, h * 64:(h + 1) * 64],
                                gath[:].rearrange("c (p k) -> c p k", k=KPAD),
                                axis=AX.X, op=ALU.max,
                            )

                        pv = pmid.tile([64, 128], FP, tag="pmid")
                        nc.tensor.matmul(pv[:], wd[li][:], Ssrc[0:Cin, bs_])
                        vT = spool.tile([64, 128], FP, tag="vT")
                        nc.scalar.activation(
                            vT[:], pv[:], ACTF.Identity, bias=bt[li][:]
                        )
                        zT = spool.tile([64, 128], FP, tag="zT")
                        nc.vector.tensor_tensor(zT[:], vT[:], mT[:], op=ALU.add)
                        nc.vector.scalar_tensor_tensor(
                            XLT_new[:, bs_], zT[:], NEG, zT[:],
                            op0=ALU.mult, op1=ALU.max,
                        )
                XLT = XLT_new

                # ---- AllGather new features ----
                ccin = dpool.tile([64, NQ], FP, tag="ccin")
                nc.sync.dma_start(ccin[:], XLT[:])
                ccout = dpool.tile(
                    [n_cores * 64, NQ], FP, tag="ccout", addr_space="Shared"
                )
                nc.gpsimd.collective_compute(
                    "AllGather",
                    ALU.bypass,
                    replica_groups=[list(range(n_cores))],
                    ins=[ccin[:].opt()],
                    outs=[ccout[:].opt()],
                )
                cur_ccout = ccout
                if li < 2:
                    STAT_new = xpool.tile([65, NQ], FP, tag="stat")
                    nc.scalar.mul(STAT_new[0:64, :], XLT[:], 2.0)
                    nc.gpsimd.memset(STAT_new[64:65, :], 1.0)
                    cur_STAT = STAT_new

            for li in range(3):
                conv_layer(li)

            # ---------------- edge head ----------------
            # x3^T full [64, N] from the final AllGather
            X3T = upool.tile([64, N], FP, tag="ut")
            for r in range(n_cores):
                nc.sync.dma_start(
                    X3T[:, r * NQ:(r + 1) * NQ],
                    cur_ccout[r * 64:(r + 1) * 64, :],
                )
            for ec in range(NECH):
                iw = ECH // 16
                g0 = gpool.tile([64, ECH], FP, tag="gath")
                nc.gpsimd.ap_gather(
                    g0[:], X3T[:], ew0t[:, ec * iw:(ec + 1) * iw],
                    channels=64, num_elems=N, d=1, num_idxs=ECH,
                )
                g1 = gpool.tile([64, ECH], FP, tag="gath")
                nc.gpsimd.ap_gather(
                    g1[:], X3T[:], ew1t[:, ec * iw:(ec + 1) * iw],
                    channels=64, num_elems=N, d=1, num_idxs=ECH,
                )
                for s in range(ECH // 512):
                    ss = slice(s * 512, (s + 1) * 512)
                    pz = pmid.tile([64, 512], FP, tag="pmid")
                    nc.tensor.matmul(
                        pz[:], what[:], g0[:, ss],
                        start=True, stop=False,
                    )
                    nc.tensor.matmul(
                        pz[:], whbt[:], g1[:, ss],
                        start=False, stop=True,
                    )
                    hE = spool.tile([64, 512], FP, tag="hE")
                    nc.scalar.activation(hE[:], pz[:], ACTF.Identity, bias=bh1t[:])
                    nc.vector.scalar_tensor_tensor(
                        hE[:], hE[:], NEG, hE[:], op0=ALU.mult, op1=ALU.max
                    )
                    po = pmid.tile([1, 512], FP, tag="pmid")
                    nc.tensor.matmul(
                        po[:], w2t[:], hE[:]
                    )
                    o512 = spool.tile([1, 512], FP, tag="o512")
                    nc.scalar.activation(o512[:], po[:], ACTF.Sigmoid, bias=bh2t[:])
                    nc.sync.dma_start(
                        out_d[ec * ECH + s * 512: ec * ECH + (s + 1) * 512],
                        o512[:],
                    )

    nc.compile()
    return nc


# ------------------------------------------------------------------
# host side
# ------------------------------------------------------------------

def prepare_inputs(x, edge_index, W1, b1, W2, b2, W3, b3, Wh1, bh1, Wh2, bh2,
                   n_cores=N_CORES):
    """Build the per-core input maps (all numpy, fp32)."""
    x = np.asarray(x, np.float32)
    N = x.shape[0]
    ei = np.asarray(edge_index)
    E = ei.shape[1]
    NQ = N // n_cores
    EC = E // n_cores
    ECH = min(2048, EC)

    xT = np.ascontiguousarray(x.T)                       # [3, N]
    sq = (x * x).sum(axis=1, dtype=np.float32)           # [N]
    m1 = np.concatenate([xT, -sq[None, :]], axis=0).astype(np.float32)

    def halfsplit(W, C):
        W = np.asarray(W, np.float32)
        return (0.5 * (W[:C] - W[C:])).astype(np.float32), np.ascontiguousarray(W[C:])

    wd1, wb1 = halfsplit(W1, 3)
    wd2, wb2 = halfsplit(W2, 64)
    wd3, wb3 = halfsplit(W3, 64)
    Wh1 = np.asarray(Wh1, np.float32)
    wha, whb = np.ascontiguousarray(Wh1[:64]), np.ascontiguousarray(Wh1[64:])

    v = np.sort(ei, axis=0)                              # canonical edges
    v0 = v[0].astype(np.int64)
    v1 = v[1].astype(np.int64)

    def wrap_idx(vals):
        # vals [EC] -> [64, EC//16] int16 wrapped per-16 within each
        # ECH-chunk, replicated across the 4 active Q7 core groups.
        segs = []
        for c in range(EC // ECH):
            seg = vals[c * ECH:(c + 1) * ECH].reshape(ECH // 16, 16).T
            segs.append(seg)
        w16 = np.concatenate(segs, axis=1).astype(np.int16)
        return np.tile(w16, (4, 1))

    common = {
        "m1": m1,
        "wd1": wd1, "wb1": wb1, "b1t": np.asarray(b1, np.float32).reshape(64, 1),
        "wd2": wd2, "wb2": wb2, "b2t": np.asarray(b2, np.float32).reshape(64, 1),
        "wd3": wd3, "wb3": wb3, "b3t": np.asarray(b3, np.float32).reshape(64, 1),
        "wha": wha, "whb": whb,
        "bh1t": np.asarray(bh1, np.float32).reshape(64, 1),
        "w2t": np.asarray(Wh2, np.float32).reshape(64, 1),
        "bh2t": np.asarray(bh2, np.float32).reshape(1, 1),
    }
    in_maps = []
    for r in range(n_cores):
        im = dict(common)
        im["stat1"] = np.concatenate(
            [2.0 * xT[:, r * NQ:(r + 1) * NQ], np.ones((1, NQ), np.float32)], axis=0
        ).astype(np.float32)
        im["ew0"] = wrap_idx(v0[r * EC:(r + 1) * EC])
        im["ew1"] = wrap_idx(v1[r * EC:(r + 1) * EC])
        in_maps.append(im)
    return in_maps


_CACHE = {}


def _get_program(N, E):
    key = (N, E)
    if key not in _CACHE:
        _CACHE[key] = build_program(N=N, E=E)
    return _CACHE[key]


def kernel(x, edge_index, W1, b1, W2, b2, W3, b3, Wh1, bh1, Wh2, bh2):
    x = np.asarray(x, np.float32)
    ei = np.asarray(edge_index)
    N, E = x.shape[0], ei.shape[1]
    nc = _get_program(N, E)
    in_maps = prepare_inputs(x, ei, W1, b1, W2, b2, W3, b3, Wh1, bh1, Wh2, bh2)
    res = run_bass_kernel_spmd(nc, in_maps, list(range(N_CORES)))
    outs = [np.asarray(res.results[i]["out"], np.float32) for i in range(N_CORES)]
    return np.concatenate(outs)



# revision 3
# speedup vs baseline: 1.4794x; 1.4794x over previous
"""Trainium2 Bass kernel for nn_ClusterPrediction (DynamicEdgeConv x3 + edge head).

Math (reference):
  3x DynamicEdgeConv: kNN(k=30) in feature space, per-edge MLP on
  [x_i, x_j - x_i] with LeakyReLU(0.2), max aggregation.
  Edge head on canonicalized (sorted) edge_index columns:
  sigmoid(w2 . LRelu(Wh1 . [x_u, x_v] + bh1) + bh2).

Device-side design (v2):
  * Distance ranking key S_ij = 2 x_i.x_j - |x_j|^2 computed as bf16
    matmuls with augmented contraction: stationary [2*x_q ; 1] (bf16),
    moving M = [x_all ; -|x|^2] (bf16, fully SBUF-resident per layer).
  * Top-30 via DVE max8 on 1024-column chunks: pack the chunk-local index
    (10 bits) into the low mantissa bits of the fp32 key, max8 -> top-8
    per chunk (16 chunks -> 128 candidates), then 4 rounds of
    max/max_index/match_replace -> ranks 1..32 (31/32 dup rank 1).
  * MLP decomposed: W.[x_i ; x_j-x_i] = (Wa-Wb).x_i + Wb.x_j; LeakyReLU
    monotone + max-commute => out_i = LRelu(v_i + b + max_k u_{j_k}),
    u = Wb.x (fp32 path for accuracy; bf16 only for ranking keys).
  * FUSED gather: u^T duplicated on both partition halves [128, N]; one
    ap_gather(channels=128) per 128-query block gathers 32 neighbors for
    queries 0..63 on partitions 0..63 and queries 64..127 on partitions
    64..127 (per-core-group wrapped indices differ) -> halves GPSIMD time,
    the true bottleneck of the baseline.
  * Per-block software pipeline: sweep(b+1) overlaps topk/wrap/gather(b).
  * AllGather split in column halves so its latency hides under the tail
    blocks of each layer.
  * Edge head: one fused gather per 2048-edge chunk produces
    [x_u ; x_v] on 128 partitions; single matmul against Wh1 [128, 64].
"""

import numpy as np

import concourse.bacc as bacc
import concourse.bass as bass
import concourse.mybir as mybir
import concourse.tile as tile
from concourse.bass_utils import run_bass_kernel_spmd

FP = mybir.dt.float32
BF = mybir.dt.bfloat16
U32 = mybir.dt.uint32
I16 = mybir.dt.int16
AX = mybir.AxisListType
ALU = mybir.AluOpType
ACTF = mybir.ActivationFunctionType

N_CORES = 8
K = 30
KPAD = 32          # 4 rounds of top-8
NEG = 0.2          # LeakyReLU slope
CH = 1024          # distance chunk columns (2 PSUM banks)
MASK_HI = 0xFFFFFC00
MASK_LO = 0x3FF


def build_program(N=16384, E=262144, n_cores=N_CORES):
    NQ = N // n_cores            # queries per core
    NB = NQ // 128               # 128-query blocks per core
    NCH = N // CH                # 1024-chunks per distance row
    NCAND = NCH * 8              # candidates per row
    EC = E // n_cores            # edges per core
    ECH = 2048                   # edges per gather chunk
    NECH = EC // ECH
    PCH = 512                    # pre-pass chunk (fp32 moving limit)
    NPCH = N // PCH
    assert NQ % 128 == 0 and N % CH == 0 and EC % ECH == 0

    nc = bacc.Bacc("TRN2", target_bir_lowering=False, num_devices=n_cores)

    # ---------------- I/O ----------------
    m1_d = nc.dram_tensor("m1", [3, N], FP, kind="ExternalInput")
    m1b_d = nc.dram_tensor("m1b", [4, N], BF, kind="ExternalInput")
    stat0_d = nc.dram_tensor("stat0", [4, NQ], BF, kind="ExternalInput")
    xloc0_d = nc.dram_tensor("xloc0", [3, NQ], FP, kind="ExternalInput")
    # wb (dup cols) for u = Wb.x ; wdf (dup cols) for v = (Wa-Wb).x
    wbd1_d = nc.dram_tensor("wbd1", [3, 128], FP, kind="ExternalInput")
    wdf1_d = nc.dram_tensor("wdf1", [3, 128], FP, kind="ExternalInput")
    btd1_d = nc.dram_tensor("btd1", [128, 1], FP, kind="ExternalInput")
    wbd2_d = nc.dram_tensor("wbd2", [64, 128], FP, kind="ExternalInput")
    wdf2_d = nc.dram_tensor("wdf2", [64, 128], FP, kind="ExternalInput")
    btd2_d = nc.dram_tensor("btd2", [128, 1], FP, kind="ExternalInput")
    wbd3_d = nc.dram_tensor("wbd3", [64, 128], FP, kind="ExternalInput")
    wdf3_d = nc.dram_tensor("wdf3", [64, 128], FP, kind="ExternalInput")
    btd3_d = nc.dram_tensor("btd3", [128, 1], FP, kind="ExternalInput")
    whc_d = nc.dram_tensor("whc", [128, 64], FP, kind="ExternalInput")
    bh1_d = nc.dram_tensor("bh1t", [64, 1], FP, kind="ExternalInput")
    w2_d = nc.dram_tensor("w2t", [64, 1], FP, kind="ExternalInput")
    bh2_d = nc.dram_tensor("bh2t", [1, 1], FP, kind="ExternalInput")
    ew_d = nc.dram_tensor("ewt", [128, EC // 16], I16, kind="ExternalInput")
    out_d = nc.dram_tensor("out", [EC], FP, kind="ExternalOutput")

    with tile.TileContext(nc, num_cores=n_cores) as tc:
        with (
            tc.tile_pool(name="const", bufs=1) as cpool,
            tc.tile_pool(name="prep", bufs=3) as ppool,
            tc.tile_pool(name="xpool", bufs=1) as xpool,
            tc.tile_pool(name="xlt", bufs=2) as xltpool,
            tc.tile_pool(name="upool", bufs=1) as upool,
            tc.tile_pool(name="mpool", bufs=1) as mpool,
            tc.tile_pool(name="cpoolc", bufs=2) as candpool,
            tc.tile_pool(name="pk", bufs=2) as pkpool,
            tc.tile_pool(name="small", bufs=3) as spool,
            tc.tile_pool(name="hpool", bufs=2) as hpool,
            tc.tile_pool(name="wrp", bufs=4) as wpool,
            tc.tile_pool(name="gat", bufs=2) as gpool,
            tc.tile_pool(name="dram", bufs=2, space="DRAM") as dpool,
            tc.tile_pool(name="pdist", bufs=3, space="PSUM") as pdist,
            tc.tile_pool(name="pmid", bufs=2, space="PSUM") as pmid,
        ):
            # ------------ load constants ------------
            def load(dram, shape, dtype=FP):
                t = cpool.tile(shape, dtype, tag=f"c_{dram.name}")
                nc.sync.dma_start(t[:], dram[:])
                return t

            stat0t = load(stat0_d, [4, NQ], BF)
            xloc0t = load(xloc0_d, [3, NQ])
            wbd = [load(wbd1_d, [3, 128]), load(wbd2_d, [64, 128]),
                   load(wbd3_d, [64, 128])]
            wdf = [load(wdf1_d, [3, 128]), load(wdf2_d, [64, 128]),
                   load(wdf3_d, [64, 128])]
            btd = [load(btd1_d, [128, 1]), load(btd2_d, [128, 1]),
                   load(btd3_d, [128, 1])]
            whct = load(whc_d, [128, 64])
            bh1t = load(bh1_d, [64, 1])
            w2t = load(w2_d, [64, 1])
            bh2t = load(bh2_d, [1, 1])
            ewt = load(ew_d, [128, EC // 16], I16)

            iota10 = cpool.tile([128, CH], U32)
            nc.gpsimd.iota(iota10[:], pattern=[[1, CH]], base=0,
                           channel_multiplier=0)
            mask_hi = cpool.tile([128, 1], U32)
            nc.gpsimd.memset(mask_hi[:], MASK_HI)
            mask_lo = cpool.tile([128, 1], U32)
            nc.gpsimd.memset(mask_lo[:], MASK_LO)
            mask_p8 = cpool.tile([128, 1], U32)
            nc.gpsimd.memset(mask_p8[:], 0xFFFFFFF8)
            ones64 = cpool.tile([64, 1], FP)
            nc.gpsimd.memset(ones64[:], 1.0)

            cur_STAT = None   # [65, NQ] bf16: rows 0..63 = 2*x^T, row 64 ones
            cur_xlt = None    # [64, NQ] fp32 local features
            cur_ccout = None  # pair of DRAM [8*64, NQ/2] (local column halves)

            def conv_layer(li):
                nonlocal cur_STAT, cur_xlt, cur_ccout
                Cin = 3 if li == 0 else 64
                KD = Cin + 1
                STATs = stat0t if li == 0 else cur_STAT
                VMOV = xloc0t if li == 0 else cur_xlt

                # ---- pre-pass: UT2 [128, N] = dup(wb^T.x^T); M [65, N] bf16
                UT2 = upool.tile([128, N], FP, tag="ut")
                MA = mpool.tile([65, N], BF, tag="ma")
                if li == 0:
                    nc.sync.dma_start(MA[0:4, :], m1b_d[:])
                for c in range(NPCH):
                    cs = slice(c * PCH, (c + 1) * PCH)
                    trowst = ppool.tile([64, PCH], FP, tag="trows")
                    if li == 0:
                        nc.sync.dma_start(trowst[0:3, :], m1_d[:, cs])
                        trows = trowst[0:3, :]
                    else:
                        g0 = c * PCH
                        rk = g0 // NQ
                        lc = g0 % NQ
                        half = lc // (NQ // 2)
                        rc = lc % (NQ // 2)
                        nc.sync.dma_start(
                            trowst[:],
                            cur_ccout[half][rk * 64:(rk + 1) * 64,
                                            rc:rc + PCH],
                        )
                        trows = trowst[:]
                    pu = pmid.tile([128, PCH], FP, tag="pmid")
                    nc.tensor.matmul(pu[:], wbd[li][:], trows)
                    nc.scalar.copy(UT2[:, cs], pu[:])
                    if li > 0:
                        # features (bf16) and -|x|^2 row for the key matrix
                        nc.vector.tensor_copy(MA[0:64, cs], trows)
                        sqc = ppool.tile([64, PCH], FP, tag="sqc")
                        nc.scalar.square(sqc[:], trows)
                        po1 = pmid.tile([1, PCH], FP, tag="pmid")
                        nc.tensor.matmul(po1[:], ones64[:], sqc[:])
                        nc.scalar.mul(MA[64:65, cs], po1[:], -1.0)

                XLTf = xpool.tile([128, NQ // 2], FP, tag="xltf")
                XLT_new = xltpool.tile([64, NQ], FP, tag="xlt")

                prev = None  # deferred finish state
                ags = []

                def finish_block(st):
                    b, gath = st
                    bs_ = slice(b * 128, (b + 1) * 128)
                    # max over 32 neighbors: [128, 64]
                    mT2 = spool.tile([128, 64], FP, tag="mT2")
                    nc.vector.tensor_reduce(
                        mT2[:],
                        gath[:].rearrange("c (p k) -> c p k", k=KPAD),
                        axis=AX.X, op=ALU.max,
                    )
                    # v = (Wa-Wb).x for all 128 queries, dup on both halves
                    pv2 = pmid.tile([128, 128], FP, tag="pmid")
                    nc.tensor.matmul(pv2[:], wdf[li][:], VMOV[0:Cin, bs_])
                    vT2 = spool.tile([128, 64], FP, tag="vT2")
                    nc.scalar.activation(
                        vT2[0:64, :], pv2[0:64, 0:64], ACTF.Identity,
                        bias=btd[li][0:64],
                    )
                    nc.scalar.activation(
                        vT2[64:128, :], pv2[64:128, 64:128], ACTF.Identity,
                        bias=btd[li][64:128],
                    )
                    zT2 = spool.tile([128, 64], FP, tag="zT2")
                    nc.vector.tensor_tensor(zT2[:], vT2[:], mT2[:], op=ALU.add)
                    nc.vector.scalar_tensor_tensor(
                        XLTf[:, b * 64:(b + 1) * 64], zT2[:], NEG, zT2[:],
                        op0=ALU.mult, op1=ALU.max,
                    )

                def unfold_half(h):
                    # blocks [h*8, h*8+8) -> XLT_new cols [h*1024, (h+1)*1024)
                    fs = slice(h * 512, (h + 1) * 512)
                    dst = XLT_new[:, h * 1024:(h + 1) * 1024].rearrange(
                        "c (b q) -> c b q", q=128
                    )
                    nc.vector.tensor_copy(
                        dst[:, :, 0:64],
                        XLTf[0:64, fs].rearrange("c (b q) -> c b q", q=64),
                    )
                    nc.sync.dma_start(
                        dst[:, :, 64:128],
                        XLTf[64:128, fs].rearrange("c (b q) -> c b q", q=64),
                    )

                def allgather_half(h):
                    ccin = dpool.tile([64, NQ // 2], FP, tag=f"ccin{h}")
                    nc.sync.dma_start(
                        ccin[:], XLT_new[:, h * (NQ // 2):(h + 1) * (NQ // 2)]
                    )
                    ccout = dpool.tile(
                        [n_cores * 64, NQ // 2], FP, tag=f"ccout{h}",
                        addr_space="Shared",
                    )
                    nc.gpsimd.collective_compute(
                        "AllGather",
                        ALU.bypass,
                        replica_groups=[list(range(n_cores))],
                        ins=[ccin[:].opt()],
                        outs=[ccout[:].opt()],
                    )
                    ags.append(ccout)

                for b in range(NB):
                    bs_ = slice(b * 128, (b + 1) * 128)
                    # ---------- distance sweep for block b ----------
                    cand = candpool.tile([128, NCAND], FP, tag="cand")
                    for c in range(NCH):
                        pd = pdist.tile([128, CH], FP, tag="pd")
                        for hh in range(2):
                            nc.tensor.matmul(
                                pd[:, hh * 512:(hh + 1) * 512],
                                STATs[0:KD, bs_],
                                MA[0:KD, c * CH + hh * 512:
                                   c * CH + (hh + 1) * 512],
                            )
                        pk = pkpool.tile([128, CH], U32, tag="pk")
                        for hh in range(2):
                            hs = slice(hh * 512, (hh + 1) * 512)
                            nc.vector.scalar_tensor_tensor(
                                pk[:, hs],
                                pd[:, hs].bitcast(U32),
                                mask_hi[:],
                                iota10[:, hs],
                                op0=ALU.bitwise_and,
                                op1=ALU.bitwise_or,
                            )
                        nc.vector.max(
                            cand[:, c * 8:(c + 1) * 8], pk[:].bitcast(FP)
                        )

                    # ---------- finish previous block ----------
                    if prev is not None:
                        finish_block(prev)
                        if b == 8:
                            unfold_half(0)
                            allgather_half(0)

                    # ---------- topk + decode + wrap + gather ----------
                    wv = spool.tile([128, KPAD], FP, tag="wv")
                    pos = spool.tile([128, KPAD], U32, tag="pos")
                    for r in range(4):
                        rs = slice(r * 8, (r + 1) * 8)
                        nc.vector.max(wv[:, rs], cand[:])
                        nc.vector.max_index(pos[:, rs], wv[:, rs], cand[:])
                        if r < 3:
                            nc.vector.match_replace(
                                cand[:], wv[:, rs], cand[:], -3.0e38
                            )
                    posm = spool.tile([128, KPAD], U32, tag="posm")
                    nc.vector.scalar_tensor_tensor(
                        posm[:], pos[:], mask_p8[:], pos[:],
                        op0=ALU.bitwise_and, op1=ALU.bypass,
                    )
                    posf = spool.tile([128, KPAD], FP, tag="posf")
                    nc.vector.tensor_copy(posf[:], posm[:])
                    lom = spool.tile([128, KPAD], U32, tag="lom")
                    nc.vector.scalar_tensor_tensor(
                        lom[:], wv[:].bitcast(U32), mask_lo[:],
                        wv[:].bitcast(U32),
                        op0=ALU.bitwise_and, op1=ALU.bypass,
                    )
                    lof = spool.tile([128, KPAD], FP, tag="lof")
                    nc.vector.tensor_copy(lof[:], lom[:])
                    idxf = spool.tile([128, KPAD], FP, tag="idxf")
                    nc.vector.scalar_tensor_tensor(
                        idxf[:], posf[:], float(CH // 8), lof[:],
                        op0=ALU.mult, op1=ALU.add,
                    )
                    # ranks 31/32 are outside the top-30: duplicate rank 1
                    nc.vector.tensor_copy(idxf[:, 30:31], idxf[:, 0:1])
                    nc.vector.tensor_copy(idxf[:, 31:32], idxf[:, 0:1])
                    idx16 = spool.tile([128, KPAD], I16, tag="idx16")
                    nc.vector.tensor_copy(idx16[:], idxf[:])

                    # wrap via DRAM bounce: queries 0..63 -> groups 0..3,
                    # queries 64..127 -> groups 4..7
                    sc = dpool.tile([128, KPAD], I16, tag="scidx")
                    nc.sync.dma_start(sc[:], idx16[:])
                    wrap = wpool.tile([128, 128], I16, tag="wrap")
                    for g in range(8):
                        src = sc[(g // 4) * 64:(g // 4) * 64 + 64, :].rearrange(
                            "p (kh q) -> q p kh", q=16
                        )
                        dst = wrap[g * 16:(g + 1) * 16, :].rearrange(
                            "q (p kh) -> q p kh", kh=2
                        )
                        nc.sync.dma_start(dst, src)

                    gath = gpool.tile([128, 64 * KPAD], FP, tag="gath")
                    nc.gpsimd.ap_gather(
                        gath[:], UT2[:], wrap[:],
                        channels=128, num_elems=N, d=1,
                        num_idxs=64 * KPAD,
                    )
                    prev = (b, gath)

                finish_block(prev)
                unfold_half(1)
                allgather_half(1)
                cur_xlt = XLT_new
                cur_ccout = (ags[0], ags[1])

                if li < 2:
                    STAT_new = xpool.tile([65, NQ], BF, tag="stat")
                    nc.scalar.mul(STAT_new[0:64, :], XLT_new[:], 2.0)
                    nc.gpsimd.memset(STAT_new[64:65, :], 1.0)
                    cur_STAT = STAT_new

            for li in range(3):
                conv_layer(li)

            # ---------------- edge head ----------------
            # x3^T full, duplicated on both partition halves [128, N]
            ag0, ag1 = cur_ccout
            X3 = upool.tile([128, N], FP, tag="ut")
            for r in range(n_cores):
                for ph in range(2):
                    ps = slice(ph * 64, (ph + 1) * 64)
                    nc.sync.dma_start(
                        X3[ps, r * NQ:r * NQ + NQ // 2],
                        ag0[r * 64:(r + 1) * 64, :],
                    )
                    nc.sync.dma_start(
                        X3[ps, r * NQ + NQ // 2:(r + 1) * NQ],
                        ag1[r * 64:(r + 1) * 64, :],
                    )
            for ec in range(NECH):
                g2 = gpool.tile([128, ECH], FP, tag="gath")
                nc.gpsimd.ap_gather(
                    g2[:], X3[:], ewt[:, ec * 128:(ec + 1) * 128],
                    channels=128, num_elems=N, d=1, num_idxs=ECH,
                )
                for s in range(ECH // 512):
                    ss = slice(s * 512, (s + 1) * 512)
                    pz = pmid.tile([64, 512], FP, tag="pmid")
                    nc.tensor.matmul(pz[:], whct[:], g2[:, ss])
                    hE = hpool.tile([64, 512], FP, tag="hE")
                    nc.scalar.activation(hE[:], pz[:], ACTF.Identity,
                                         bias=bh1t[:])
                    nc.vector.scalar_tensor_tensor(
                        hE[:], hE[:], NEG, hE[:], op0=ALU.mult, op1=ALU.max
                    )
                    po = pmid.tile([1, 512], FP, tag="pmid")
                    nc.tensor.matmul(po[:], w2t[:], hE[:])
                    o512 = hpool.tile([1, 512], FP, tag="o512")
                    nc.scalar.activation(o512[:], po[:], ACTF.Sigmoid,
                                         bias=bh2t[:])
                    nc.sync.dma_start(
                        out_d[ec * ECH + s * 512:ec * ECH + (s + 1) * 512],
                        o512[:],
                    )

    nc.compile()
    return nc


# ------------------------------------------------------------------
# host side
# ------------------------------------------------------------------

def prepare_inputs(x, edge_index, W1, b1, W2, b2, W3, b3, Wh1, bh1, Wh2, bh2,
                   n_cores=N_CORES):
    """Build the per-core input maps (all numpy)."""
    from ml_dtypes import bfloat16

    x = np.asarray(x, np.float32)
    N = x.shape[0]
    ei = np.asarray(edge_index)
    E = ei.shape[1]
    NQ = N // n_cores
    EC = E // n_cores
    ECH = 2048

    xT = np.ascontiguousarray(x.T)                       # [3, N]
    sq = (x * x).sum(axis=1, dtype=np.float32)           # [N]
    m1 = xT.astype(np.float32)
    m1b = np.concatenate([xT, -sq[None, :]], axis=0).astype(bfloat16)

    def dup(W):
        W = np.asarray(W, np.float32)
        return np.ascontiguousarray(np.concatenate([W, W], axis=1),
                                    dtype=np.float32)

    def split(W, C):
        W = np.asarray(W, np.float32)
        return W[:C], W[C:]

    wa1, wb1 = split(W1, 3)
    wa2, wb2 = split(W2, 64)
    wa3, wb3 = split(W3, 64)

    def bdup(b):
        b = np.asarray(b, np.float32).reshape(64, 1)
        return np.concatenate([b, b], axis=0).astype(np.float32)

    v = np.sort(ei, axis=0)                              # canonical edges
    v0 = v[0].astype(np.int64)
    v1 = v[1].astype(np.int64)

    def wrap_edges(vals):
        # vals [EC] -> [16, EC//16]: w[q, ec*128+m] = vals[ec*2048+m*16+q]
        segs = vals.reshape(EC // ECH, ECH // 16, 16).transpose(0, 2, 1)
        return np.concatenate(list(segs), axis=1).astype(np.int16)

    common = {
        "m1": m1, "m1b": m1b,
        "wbd1": dup(wb1), "wdf1": dup(wa1 - wb1), "btd1": bdup(b1),
        "wbd2": dup(wb2), "wdf2": dup(wa2 - wb2), "btd2": bdup(b2),
        "wbd3": dup(wb3), "wdf3": dup(wa3 - wb3), "btd3": bdup(b3),
        "whc": np.asarray(Wh1, np.float32),
        "bh1t": np.asarray(bh1, np.float32).reshape(64, 1),
        "w2t": np.asarray(Wh2, np.float32).reshape(64, 1),
        "bh2t": np.asarray(bh2, np.float32).reshape(1, 1),
    }
    in_maps = []
    for r in range(n_cores):
        im = dict(common)
        xl = xT[:, r * NQ:(r + 1) * NQ]
        im["stat0"] = np.concatenate(
            [2.0 * xl, np.ones((1, NQ), np.float32)], axis=0
        ).astype(bfloat16)
        im["xloc0"] = np.ascontiguousarray(xl, dtype=np.float32)
        w0 = wrap_edges(v0[r * EC:(r + 1) * EC])
        w1 = wrap_edges(v1[r * EC:(r + 1) * EC])
        im["ewt"] = np.ascontiguousarray(
            np.concatenate([np.tile(w0, (4, 1)), np.tile(w1, (4, 1))], axis=0),
            dtype=np.int16,
        )
        in_maps.append(im)
    return in_maps


_CACHE = {}


def _get_program(N, E):
    key = (N, E)
    if key not in _CACHE:
        _CACHE[key] = build_program(N=N, E=E)
    return _CACHE[key]


def kernel(x, edge_index, W1, b1, W2, b2, W3, b3, Wh1, bh1, Wh2, bh2):
    x = np.asarray(x, np.float32)
    ei = np.asarray(edge_index)
    N, E = x.shape[0], ei.shape[1]
    nc = _get_program(N, E)
    in_maps = prepare_inputs(x, ei, W1, b1, W2, b2, W3, b3, Wh1, bh1, Wh2, bh2)
    res = run_bass_kernel_spmd(nc, in_maps, list(range(N_CORES)))
    outs = [np.asarray(res.results[i]["out"], np.float32) for i in range(N_CORES)]
    return np.concatenate(outs)
